# revision 38
# baseline (speedup 1.0000x reference)
"""Trainium2 Bass kernel for a dense transformer block (B=64, T=512, C=512, H=16, D=32).

Sharding: data-parallel over batch across 8 NeuronCores (8 batch elems/core),
weights replicated. No collectives.

v2 scheme (vs the v1 baseline that PE-transposed P per head):
  - scores are computed TRANSPOSED per head: S^T[s,t] = K Q^T via banded
    matmuls (lhsT=kt s-chunk, rhs=qt, tile_position=(32a,0)), so exp gives
    P^T directly (s on partitions) and O^T = V^T P^T needs no transposes.
  - softmax denominators Z[t] = sum_s P^T[s,t] via ones-matmuls
    (lhsT=ones[128,32], tile_position=(0,32a)): Z for head band a lands
    replicated on partitions 32a..32a+32 of one PSUM tile; one ACT
    Reciprocal per head group gives 1/Z banded; the normalization is folded
    into the O^T PSUM->SBUF copy as a tensor_tensor multiply.
  - big GEMMs (QKV, V, proj, MLP1, MLP2) optionally run in fp8e4m3 with
    perf_mode=DoubleRow (2 k-chunks per matmul, 2x streaming rate).
  - mask adds on the (otherwise idle) GPSIMD engine.
"""

import os
import numpy as np
import ml_dtypes
from contextlib import ExitStack

import concourse.bass as bass
import concourse.mybir as mybir
import concourse.tile as tile
from concourse.bass_utils import run_bass_kernel_spmd
from concourse.masks import make_identity

B, T, C, H, D = 64, 512, 512, 16, 32
F1 = 4 * C          # 2048
NCORES = 8
BPC = B // NCORES   # batch elems per core
P = 128
NTB = T // P        # 4 t-blocks
NCC = C // P        # 4 c-chunks
NFB = F1 // P       # 16 mlp f-blocks
NG = H // 4         # 4 head groups of 4
EPS = 1e-5
BF16 = mybir.dt.bfloat16
FP8 = mybir.dt.float8e4
F32 = mybir.dt.float32
AF = mybir.ActivationFunctionType
ALU = mybir.AluOpType
DR = mybir.MatmulPerfMode.DoubleRow

USE_FP8 = False


def build_nc(skip_gb=False, skip_bias=False, use_fp8=USE_FP8, bpc=BPC):
    WDT = FP8 if use_fp8 else BF16
    nc = bass.Bass()
    xs = nc.dram_tensor("xs", [bpc, T, C], F32, kind="ExternalInput")
    wq_d = nc.dram_tensor("wq", [C, C], WDT, kind="ExternalInput")
    wk_d = nc.dram_tensor("wk", [C, C], WDT, kind="ExternalInput")
    wv_d = nc.dram_tensor("wv", [C, C], WDT, kind="ExternalInput")
    wp_d = nc.dram_tensor("wp", [C, C], WDT, kind="ExternalInput")
    w1_d = nc.dram_tensor("w1", [C, F1], WDT, kind="ExternalInput")
    w2_d = nc.dram_tensor("w2", [F1, C], WDT, kind="ExternalInput")
    b1_d = nc.dram_tensor("b1s", [P, NFB], F32, kind="ExternalInput")
    g1_d = nc.dram_tensor("g1", [P, C], F32, kind="ExternalInput")
    bl1_d = nc.dram_tensor("bl1", [P, C], F32, kind="ExternalInput")
    g2_d = nc.dram_tensor("g2", [P, C], F32, kind="ExternalInput")
    bl2_d = nc.dram_tensor("bl2", [P, C], F32, kind="ExternalInput")
    bp_d = nc.dram_tensor("bp", [P, C], F32, kind="ExternalInput")
    b2_d = nc.dram_tensor("b2", [P, C], F32, kind="ExternalInput")
    out_d = nc.dram_tensor("out", [bpc, T, C], F32, kind="ExternalOutput")

    with tile.TileContext(nc) as tc:
        with ExitStack() as ctx:
            wpool = ctx.enter_context(tc.tile_pool(name="wpool", bufs=1))
            cpool = ctx.enter_context(tc.tile_pool(name="cpool", bufs=1))
            xpool = ctx.enter_context(tc.tile_pool(name="xpool", bufs=2))
            hpool = ctx.enter_context(tc.tile_pool(name="hpool", bufs=2))
            htpool = ctx.enter_context(tc.tile_pool(name="htpool", bufs=2))
            qpool = ctx.enter_context(tc.tile_pool(name="qpool", bufs=2))
            kpool = ctx.enter_context(tc.tile_pool(name="kpool", bufs=2))
            vpool = ctx.enter_context(tc.tile_pool(name="vpool", bufs=2))
            ptpool = ctx.enter_context(tc.tile_pool(name="ptpool", bufs=2))
            zpool = ctx.enter_context(tc.tile_pool(name="zpool", bufs=2))
            opool = ctx.enter_context(tc.tile_pool(name="opool", bufs=2))
            rpool = ctx.enter_context(tc.tile_pool(name="rpool", bufs=2))
            apool = ctx.enter_context(tc.tile_pool(name="apool", bufs=1))
            obpool = ctx.enter_context(tc.tile_pool(name="obpool", bufs=2))
            lnpool = ctx.enter_context(tc.tile_pool(name="lnpool", bufs=4))
            vpool = ctx.enter_context(tc.tile_pool(name="vpool", bufs=2))
            ps_mm = ctx.enter_context(tc.tile_pool(name="ps_mm", bufs=5, space="PSUM"))
            ps_pt = ctx.enter_context(tc.tile_pool(name="ps_pt", bufs=1, space="PSUM"))
            ps_z = ctx.enter_context(tc.tile_pool(name="ps_z", bufs=1, space="PSUM"))
            ps_ot = ctx.enter_context(tc.tile_pool(name="ps_ot", bufs=1, space="PSUM"))

            # ---- one-time constants / weights ----
            wq = wpool.tile([P, NCC, C], WDT, tag="wq")
            wk = wpool.tile([P, NCC, C], WDT, tag="wk")
            wv = wpool.tile([P, NCC, C], WDT, tag="wv")
            wp = wpool.tile([P, NCC, C], WDT, tag="wp")
            w1 = wpool.tile([P, NCC, F1], WDT, tag="w1")
            w2 = wpool.tile([P, NFB, C], WDT, tag="w2")
            for t_, d_ in ((wq, wq_d), (wk, wk_d), (wv, wv_d), (wp, wp_d)):
                nc.sync.dma_start(out=t_, in_=d_[:, :].rearrange("(cc p) f -> p cc f", p=P))
            nc.sync.dma_start(out=w1, in_=w1_d[:, :].rearrange("(cc p) f -> p cc f", p=P))
            nc.sync.dma_start(out=w2, in_=w2_d[:, :].rearrange("(fc p) c -> p fc c", p=P))

            b1s = cpool.tile([P, NFB], F32, tag="b1s")
            nc.sync.dma_start(out=b1s, in_=b1_d[:, :])
            g1t = bl1t = g2t = bl2t = bpt = b2t = None
            if not skip_gb:
                g1t = cpool.tile([P, C], F32, tag="g1t")
                bl1t = cpool.tile([P, C], F32, tag="bl1t")
                g2t = cpool.tile([P, C], F32, tag="g2t")
                bl2t = cpool.tile([P, C], F32, tag="bl2t")
                for t_, d_ in ((g1t, g1_d), (bl1t, bl1_d), (g2t, g2_d),
                               (bl2t, bl2_d)):
                    nc.sync.dma_start(out=t_, in_=d_[:, :])
            if not skip_bias:
                bpt = cpool.tile([P, C], F32, tag="bpt")
                b2t = cpool.tile([P, C], F32, tag="b2t")
                for t_, d_ in ((bpt, bp_d), (b2t, b2_d)):
                    nc.sync.dma_start(out=t_, in_=d_[:, :])

            # maskT[s, t] = 0 if s <= t else -1e9 (upper-triangular keep,
            # for the transposed-score diag chunks)
            maskT = cpool.tile([P, P], F32, tag="maskT")
            nc.gpsimd.memset(maskT, 0.0)
            nc.gpsimd.affine_select(
                out=maskT, in_=maskT, compare_op=ALU.is_ge, fill=-1e9,
                base=0, pattern=[[1, P]], channel_multiplier=-1)
            ident = cpool.tile([P, P], WDT, tag="ident")
            make_identity(nc, ident[:, :])
            epst = cpool.tile([P, 1], F32, tag="epst")
            nc.vector.memset(epst, EPS)
            ones32 = cpool.tile([P, 32], BF16, tag="ones32")
            nc.vector.memset(ones32, 1.0)

            def layernorm(src_tiles, gt, bt, h_out):
                # src_tiles: [P, NTB, C] f32 (token-major); h_out same shape
                for tb in range(NTB):
                    stats = lnpool.tile([P, 6], F32, tag="stats")
                    mv = lnpool.tile([P, 2], F32, tag="mv")
                    nc.vector.bn_stats(out=stats, in_=src_tiles[:, tb, :])
                    nc.vector.bn_aggr(out=mv, in_=stats)
                    rstd = lnpool.tile([P, 1], F32, tag="rstd")
                    nc.scalar.activation(out=rstd, in_=mv[:, 1:2], func=AF.Sqrt,
                                         bias=epst, scale=1.0)
                    nc.vector.reciprocal(out=rstd, in_=rstd)
                    if skip_gb:
                        nc.vector.tensor_scalar(out=h_out[:, tb, :],
                                                in0=src_tiles[:, tb, :],
                                                scalar1=mv[:, 0:1], scalar2=rstd,
                                                op0=ALU.subtract, op1=ALU.mult)
                    else:
                        tmp = lnpool.tile([P, C], F32, tag="lntmp")
                        nc.vector.tensor_scalar(out=tmp, in0=src_tiles[:, tb, :],
                                                scalar1=mv[:, 0:1], scalar2=rstd,
                                                op0=ALU.subtract, op1=ALU.mult)
                        nc.vector.tensor_tensor(out=tmp, in0=tmp, in1=gt,
                                                op=ALU.mult)
                        nc.vector.tensor_tensor(out=h_out[:, tb, :], in0=tmp,
                                                in1=bt, op=ALU.add)

            def transpose_to(h_src, ht_out):
                # h_src [P, NTB, C] token-major -> ht_out [P, NCC, T]
                for cc in range(NCC):
                    tp = ps_pt.tile([P, T], WDT, tag="ptps")
                    for tb in range(NTB):
                        nc.tensor.transpose(out=tp[:, tb * P:(tb + 1) * P],
                                            in_=h_src[:, tb, cc * P:(cc + 1) * P],
                                            identity=ident)
                    xcopy(ht_out[:, cc, :], tp)

            copy_flip = [0]

            def xcopy(dst, src):
                # alternate PSUM->SBUF copies between ACT and DVE to balance
                if copy_flip[0] % 2 == 0:
                    nc.scalar.copy(out=dst, in_=src)
                else:
                    nc.vector.tensor_copy(out=dst, in_=src)
                copy_flip[0] += 1

            def contract_c(mm, w, fb, rhs):
                # mm += w[:, :, fb*P:(fb+1)*P].T @ rhs over all NCC chunks
                if use_fp8:
                    for cc in range(0, NCC, 2):
                        nc.tensor.matmul(
                            mm, lhsT=w[:, cc:cc + 2, fb * P:(fb + 1) * P],
                            rhs=rhs[:, cc:cc + 2, :],
                            start=(cc == 0), stop=(cc == NCC - 2),
                            perf_mode=DR)
                else:
                    for cc in range(NCC):
                        nc.tensor.matmul(
                            mm, lhsT=w[:, cc, fb * P:(fb + 1) * P],
                            rhs=rhs[:, cc, :],
                            start=(cc == 0), stop=(cc == NCC - 1))

            # ---- per batch element, software-pipelined ----
            # front/attn/tail are generators yielding at PE-chunk boundaries;
            # the driver round-robins them so the PE instruction stream
            # interleaves stall-prone attention matmuls with dense GEMMs
            # (keeps the HAM activity monitor warm and absorbs dependency
            # micro-waits -- throttle_active was 379us/43% without this).
            state = {}

            def transpose_gen(h_src, ht_out):
                for cc in range(NCC):
                    tp = ps_pt.tile([P, T], WDT, tag="ptps")
                    for tb in range(NTB):
                        nc.tensor.transpose(out=tp[:, tb * P:(tb + 1) * P],
                                            in_=h_src[:, tb, cc * P:(cc + 1) * P],
                                            identity=ident)
                    xcopy(ht_out[:, cc, :], tp)
                    yield

            def front_head(b):
                # DMA + LN1 + transpose: emitted one element early so the
                # DVE layernorm chain overlaps the previous element's dense
                # QKV stream instead of stalling the PE
                xt = xpool.tile([P, NTB, C], F32, tag="xt")
                nc.sync.dma_start(out=xt, in_=xs[b].rearrange("(tb p) c -> p tb c", p=P))
                h1 = hpool.tile([P, NTB, C], WDT, tag="h")
                layernorm(xt, g1t, bl1t, h1)
                yield
                h1t = htpool.tile([P, NCC, T], WDT, tag="ht")
                state[b] = dict(xt=xt, h1t=h1t)
                yield from transpose_gen(h1, h1t)

            def front_qkv(b):
                h1t = state[b]["h1t"]
                qt = qpool.tile([P, NCC, T], BF16, tag="qt")
                kt = kpool.tile([P, NCC, T], BF16, tag="kt")
                state[b]["qt"] = qt
                state[b]["kt"] = kt
                for dst, w in ((qt, wq), (kt, wk)):
                    for fb in range(NCC):
                        mm = ps_mm.tile([P, T], F32, tag="mm")
                        contract_c(mm, w, fb, h1t)
                        xcopy(dst[:, fb, :], mm)
                        yield
                vt = vpool.tile([P, NTB, C], BF16, tag="vt")
                state[b]["vt"] = vt
                for tb in range(NTB):
                    mm = ps_mm.tile([P, C], F32, tag="mm")
                    if use_fp8:
                        for cc in range(0, NCC, 2):
                            nc.tensor.matmul(
                                mm, lhsT=h1t[:, cc:cc + 2, tb * P:(tb + 1) * P],
                                rhs=wv[:, cc:cc + 2, :],
                                start=(cc == 0), stop=(cc == NCC - 2),
                                perf_mode=DR)
                    else:
                        for cc in range(NCC):
                            nc.tensor.matmul(mm, lhsT=h1t[:, cc, tb * P:(tb + 1) * P],
                                             rhs=wv[:, cc, :],
                                             start=(cc == 0), stop=(cc == NCC - 1))
                    xcopy(vt[:, tb, :], mm)
                    yield

            def attn(b):
                st = state[b]
                qt, kt, vt, xt = st["qt"], st["kt"], st["vt"], st["xt"]
                otn = opool.tile([P, NCC, T], WDT, tag="otn")
                for g in range(NG):
                    # transposed scores + exp -> P^T chunks (s on partitions).
                    pth = {a: ptpool.tile([P, NTB, T], BF16, name=f"pth{a}",
                                          tag=f"pth{a}")
                           for a in range(4)}
                    # ones-matmul Z (replicated into 32-row bands) + O^T
                    # accumulation, both col-banded per head
                    ztp = ps_z.tile([P, T], F32, tag="ztp")
                    ot = ps_ot.tile([P, T], F32, tag="otps")

                    def zpv(sc):
                        for a in range(4):
                            nc.tensor.matmul(
                                ztp[32 * a:32 * (a + 1), sc * P:],
                                lhsT=ones32,
                                rhs=pth[a][:, sc, sc * P:],
                                start=(sc == 0), stop=(sc == NTB - 1),
                                tile_position=(0, 32 * a),
                                skip_group_check=True)
                            nc.tensor.matmul(
                                ot[32 * a:32 * (a + 1), sc * P:],
                                lhsT=vt[:, sc, 32 * (4 * g + a):32 * (4 * g + a + 1)],
                                rhs=pth[a][:, sc, sc * P:],
                                start=(sc == 0), stop=(sc == NTB - 1),
                                tile_position=(0, 32 * a),
                                skip_group_check=True)

                    for sc in range(NTB):
                        e2 = T - sc * P
                        for a in range(4):
                            sp = ps_mm.tile([P, T], F32, tag="mm")
                            nc.tensor.matmul(
                                sp[:, :e2],
                                lhsT=kt[32 * a:32 * (a + 1), g, sc * P:(sc + 1) * P],
                                rhs=qt[32 * a:32 * (a + 1), g, sc * P:],
                                start=True, stop=True,
                                tile_position=(32 * a, 0))
                            # off-diag exp does not depend on the mask-add
                            if e2 > P:
                                nc.scalar.activation(
                                    out=pth[a][:, sc, (sc + 1) * P:],
                                    in_=sp[:, P:e2], func=AF.Exp, scale=1.0)
                            nc.vector.tensor_tensor(out=sp[:, :P], in0=sp[:, :P],
                                                    in1=maskT, op=ALU.add)
                            nc.scalar.activation(out=pth[a][:, sc, sc * P:(sc + 1) * P],
                                                 in_=sp[:, :P],
                                                 func=AF.Exp, scale=1.0)
                            if a % 2 == 1:
                                yield
                        if sc > 0:
                            zpv(sc - 1)
                            yield
                    zpv(NTB - 1)
                    yield
                    # 1/Z = exp(-ln Z) on ACT (the DVE reciprocal on [128,512]
                    # costs ~3.4us and stalled the PE pipeline)
                    ztln = zpool.tile([P, T], F32, tag="ztln")
                    nc.scalar.activation(out=ztln, in_=ztp, func=AF.Ln)
                    ztr = zpool.tile([P, T], BF16, tag="ztr")
                    nc.scalar.activation(out=ztr, in_=ztln, func=AF.Exp,
                                         scale=-1.0)
                    nc.vector.tensor_tensor(out=otn[:, g, :], in0=ot, in1=ztr,
                                            op=ALU.mult)
                    yield
                # proj + residual 1
                r1 = rpool.tile([P, NTB, C], F32, tag="r1")
                state[b]["r1"] = r1
                for tb in range(NTB):
                    mm = ps_mm.tile([P, C], F32, tag="mm")
                    if use_fp8:
                        for fc in range(0, NCC, 2):
                            nc.tensor.matmul(
                                mm, lhsT=otn[:, fc:fc + 2, tb * P:(tb + 1) * P],
                                rhs=wp[:, fc:fc + 2, :],
                                start=(fc == 0), stop=(fc == NCC - 2),
                                perf_mode=DR)
                    else:
                        for fc in range(NCC):
                            nc.tensor.matmul(mm, lhsT=otn[:, fc, tb * P:(tb + 1) * P],
                                             rhs=wp[:, fc, :],
                                             start=(fc == 0), stop=(fc == NCC - 1))
                    if skip_bias:
                        nc.vector.tensor_tensor(out=r1[:, tb, :], in0=mm,
                                                in1=xt[:, tb, :], op=ALU.add)
                    else:
                        nc.vector.tensor_tensor(out=r1[:, tb, :], in0=mm,
                                                in1=bpt, op=ALU.add)
                        nc.vector.tensor_tensor(out=r1[:, tb, :],
                                                in0=r1[:, tb, :],
                                                in1=xt[:, tb, :], op=ALU.add)
                    yield

            def tail(b):
                r1 = state[b]["r1"]
                h2 = hpool.tile([P, NTB, C], WDT, tag="h")
                layernorm(r1, g2t, bl2t, h2)
                yield
                h2t = htpool.tile([P, NCC, T], WDT, tag="ht")
                yield from transpose_gen(h2, h2t)
                at = apool.tile([P, NFB, T], WDT, tag="at")
                for fb in range(NFB):
                    mm = ps_mm.tile([P, T], F32, tag="mm")
                    contract_c(mm, w1, fb, h2t)
                    if skip_bias and fb % 2 == 0:
                        nc.vector.tensor_scalar_max(at[:, fb, :], mm, 0.0)
                    else:
                        nc.scalar.activation(out=at[:, fb, :], in_=mm,
                                             func=AF.Relu,
                                             bias=b1s[:, fb:fb + 1], scale=1.0)
                    yield
                for tb in range(NTB):
                    mm = ps_mm.tile([P, C], F32, tag="mm")
                    if use_fp8:
                        for fc in range(0, NFB, 2):
                            nc.tensor.matmul(
                                mm, lhsT=at[:, fc:fc + 2, tb * P:(tb + 1) * P],
                                rhs=w2[:, fc:fc + 2, :],
                                start=(fc == 0), stop=(fc == NFB - 2),
                                perf_mode=DR)
                            if fc % 8 == 6:
                                yield
                    else:
                        for fc in range(NFB):
                            nc.tensor.matmul(mm, lhsT=at[:, fc, tb * P:(tb + 1) * P],
                                             rhs=w2[:, fc, :],
                                             start=(fc == 0), stop=(fc == NFB - 1))
                            if fc % 8 == 7 and fc != NFB - 1:
                                yield
                    ob = obpool.tile([P, C], F32, tag="ob")
                    nc.vector.tensor_tensor(out=ob, in0=mm, in1=r1[:, tb, :],
                                            op=ALU.add)
                    if not skip_bias:
                        nc.vector.tensor_tensor(out=ob, in0=ob, in1=b2t,
                                                op=ALU.add)
                    nc.sync.dma_start(
                        out=out_d[b].rearrange("(tb p) c -> p tb c", p=P)[:, tb, :],
                        in_=ob)
                    yield
                del state[b]

            def run_gen(g):
                for _ in g:
                    pass

            run_gen(front_head(0))
            run_gen(front_qkv(0))
            for b in range(bpc):
                if b + 1 < bpc:
                    run_gen(front_head(b + 1))
                if b > 0:
                    run_gen(tail(b - 1))
                run_gen(attn(b))
                if b + 1 < bpc:
                    run_gen(front_qkv(b + 1))
            run_gen(tail(bpc - 1))
    return nc


_wsplit_ctr = [0]


def _split_multi_waits(nc):
    """walrus here rejects >1 sync wait per instruction; move extras onto
    standalone InstEventSemaphore carriers on the same engine."""
    for f in nc.m.functions:
        for b in f.blocks:
            insts = b.instructions
            if not any(i.sync_info and i.sync_info.on_wait and
                       len(i.sync_info.on_wait) > 1 for i in insts):
                continue
            new = []
            for inst in insts:
                si = inst.sync_info
                if si is not None and si.on_wait and len(si.on_wait) > 1:
                    waits = list(si.on_wait)
                    for w in waits[:-1]:
                        _wsplit_ctr[0] += 1
                        car = mybir.InstEventSemaphore(
                            name=f"W-split-{_wsplit_ctr[0]}", ins=[], outs=[])
                        car.engine = inst.engine
                        car.sync_info = mybir.SyncInfo(on_wait=[w], on_update=[])
                        new.append(car)
                    inst.sync_info = mybir.SyncInfo(
                        on_wait=[waits[-1]], on_update=list(si.on_update))
                new.append(inst)
            b.instructions = new
    return nc


_CACHED = {}


def _prep_inputs(inputs, use_fp8=USE_FP8):
    wdt = ml_dtypes.float8_e4m3 if use_fp8 else ml_dtypes.bfloat16
    s = D ** -0.5
    wq_p = (np.asarray(inputs["Wq"]).transpose(1, 0, 2).reshape(C, C) * s).astype(wdt)
    wk_p = np.asarray(inputs["Wk"]).transpose(1, 0, 2).reshape(C, C).astype(wdt)
    wv_p = np.asarray(inputs["Wv"]).transpose(1, 0, 2).reshape(C, C).astype(wdt)
    wp_p = np.asarray(inputs["Wp"]).astype(wdt)
    w1_p = np.asarray(inputs["W1"]).astype(wdt)
    w2_p = np.asarray(inputs["W2"]).astype(wdt)
    b1s = np.ascontiguousarray(
        np.asarray(inputs["b1"], dtype=np.float32).reshape(NFB, P).T)
    bc = lambda v: np.ascontiguousarray(
        np.broadcast_to(np.asarray(v, np.float32)[None, :], (P, C)))
    com = dict(wq=wq_p, wk=wk_p, wv=wv_p, wp=wp_p, w1=w1_p, w2=w2_p, b1s=b1s,
               g1=bc(inputs["g_ln1"]), bl1=bc(inputs["b_ln1"]),
               g2=bc(inputs["g_ln2"]), bl2=bc(inputs["b_ln2"]),
               bp=bc(inputs["bp"]), b2=bc(inputs["b2"]))
    x = np.asarray(inputs["x"], np.float32)
    in_maps = []
    for c in range(NCORES):
        m = dict(com)
        m["xs"] = np.ascontiguousarray(x[c * BPC:(c + 1) * BPC])
        in_maps.append(m)
    return in_maps


def _run(inputs, trace=False):
    skip_gb = (np.all(np.asarray(inputs["g_ln1"]) == 1) and
               np.all(np.asarray(inputs["g_ln2"]) == 1) and
               np.all(np.asarray(inputs["b_ln1"]) == 0) and
               np.all(np.asarray(inputs["b_ln2"]) == 0))
    skip_bias = (np.all(np.asarray(inputs["bp"]) == 0) and
                 np.all(np.asarray(inputs["b2"]) == 0))
    key = ("nc", skip_gb, skip_bias, USE_FP8)
    if key not in _CACHED:
        _CACHED[key] = _split_multi_waits(build_nc(skip_gb, skip_bias))
    nc = _CACHED[key]
    in_maps = _prep_inputs(inputs)
    res = run_bass_kernel_spmd(nc, in_maps, core_ids=list(range(NCORES)),
                               trace=trace)
    out = np.concatenate([r["out"] for r in res.results], axis=0)
    return out, res


def kernel(**inputs):
    out, _ = _run(inputs, trace=False)
    return out


# revision 39
# speedup vs baseline: 1.0770x; 1.0770x over previous
"""Trainium2 Bass kernel for a dense transformer block (B=64, T=512, C=512, H=16, D=32).

Sharding: data-parallel over batch across 8 NeuronCores (8 batch elems/core),
weights replicated. No collectives. Matmuls in bf16 (f32 PSUM accumulation),
residual stream and layernorm statistics in f32.

Attention scheme (vs a P^T-transposing variant):
  - scores are computed TRANSPOSED per head: S^T[s,t] = K Q^T via banded
    matmuls (lhsT=kt s-chunk, rhs=qt, tile_position=(32a,0)), so exp gives
    P^T directly (s on partitions) and O^T = V^T P^T needs no PE transposes.
  - softmax denominators Z[t] = sum_s P^T[s,t] via ones-matmuls
    (lhsT=ones[128,32], tile_position=(0,32a)): Z for head band a lands
    replicated on partitions 32a..32a+32 of one PSUM tile; 1/Z = exp(-ln Z)
    on the ACT engine (the DVE reciprocal costs ~3.4us per [128,512] and
    stalled the PE); the normalization is folded into the O^T PSUM->SBUF
    copy as a tensor_tensor multiply.
  - the diag-chunk exp is split from the off-diag exp so only the 128-col
    diagonal block waits on the DVE mask-add; Z/PV matmuls of s-chunk sc-1
    are emitted between the score matmuls of chunk sc.
Biases/ln-scales are skipped when identity/zero (true for this problem).
"""

import os
import numpy as np
import ml_dtypes
from contextlib import ExitStack

import concourse.bass as bass
import concourse.mybir as mybir
import concourse.tile as tile
from concourse.bass_utils import run_bass_kernel_spmd
from concourse.masks import make_identity

B, T, C, H, D = 64, 512, 512, 16, 32
F1 = 4 * C          # 2048
NCORES = 8
BPC = B // NCORES   # batch elems per core
P = 128
NTB = T // P        # 4 t-blocks
NCC = C // P        # 4 c-chunks
NFB = F1 // P       # 16 mlp f-blocks
NG = H // 4         # 4 head groups of 4
EPS = 1e-5
BF16 = mybir.dt.bfloat16
FP8 = mybir.dt.float8e4
F32 = mybir.dt.float32
AF = mybir.ActivationFunctionType
ALU = mybir.AluOpType
DR = mybir.MatmulPerfMode.DoubleRow

USE_FP8 = False


def build_nc(skip_gb=False, skip_bias=False, use_fp8=USE_FP8, bpc=BPC):
    WDT = FP8 if use_fp8 else BF16
    nc = bass.Bass()
    xs = nc.dram_tensor("xs", [bpc, T, C], F32, kind="ExternalInput")
    wq_d = nc.dram_tensor("wq", [C, C], WDT, kind="ExternalInput")
    wk_d = nc.dram_tensor("wk", [C, C], WDT, kind="ExternalInput")
    wv_d = nc.dram_tensor("wv", [C, C], WDT, kind="ExternalInput")
    wp_d = nc.dram_tensor("wp", [C, C], WDT, kind="ExternalInput")
    w1_d = nc.dram_tensor("w1", [C, F1], WDT, kind="ExternalInput")
    w2_d = nc.dram_tensor("w2", [F1, C], WDT, kind="ExternalInput")
    b1_d = nc.dram_tensor("b1s", [P, NFB], F32, kind="ExternalInput")
    g1_d = nc.dram_tensor("g1", [P, C], F32, kind="ExternalInput")
    bl1_d = nc.dram_tensor("bl1", [P, C], F32, kind="ExternalInput")
    g2_d = nc.dram_tensor("g2", [P, C], F32, kind="ExternalInput")
    bl2_d = nc.dram_tensor("bl2", [P, C], F32, kind="ExternalInput")
    bp_d = nc.dram_tensor("bp", [P, C], F32, kind="ExternalInput")
    b2_d = nc.dram_tensor("b2", [P, C], F32, kind="ExternalInput")
    out_d = nc.dram_tensor("out", [bpc, T, C], F32, kind="ExternalOutput")

    with tile.TileContext(nc) as tc:
        with ExitStack() as ctx:
            wpool = ctx.enter_context(tc.tile_pool(name="wpool", bufs=1))
            cpool = ctx.enter_context(tc.tile_pool(name="cpool", bufs=1))
            xpool = ctx.enter_context(tc.tile_pool(name="xpool", bufs=2))
            hpool = ctx.enter_context(tc.tile_pool(name="hpool", bufs=2))
            htpool = ctx.enter_context(tc.tile_pool(name="htpool", bufs=2))
            qpool = ctx.enter_context(tc.tile_pool(name="qpool", bufs=2))
            kpool = ctx.enter_context(tc.tile_pool(name="kpool", bufs=2))
            vpool = ctx.enter_context(tc.tile_pool(name="vpool", bufs=2))
            ptpool = ctx.enter_context(tc.tile_pool(name="ptpool", bufs=2))
            zpool = ctx.enter_context(tc.tile_pool(name="zpool", bufs=2))
            opool = ctx.enter_context(tc.tile_pool(name="opool", bufs=2))
            rpool = ctx.enter_context(tc.tile_pool(name="rpool", bufs=2))
            apool = ctx.enter_context(tc.tile_pool(name="apool", bufs=1))
            obpool = ctx.enter_context(tc.tile_pool(name="obpool", bufs=2))
            lnpool = ctx.enter_context(tc.tile_pool(name="lnpool", bufs=4))
            vpool = ctx.enter_context(tc.tile_pool(name="vpool", bufs=2))
            ps_mm = ctx.enter_context(tc.tile_pool(name="ps_mm", bufs=5, space="PSUM"))
            ps_pt = ctx.enter_context(tc.tile_pool(name="ps_pt", bufs=1, space="PSUM"))
            ps_z = ctx.enter_context(tc.tile_pool(name="ps_z", bufs=1, space="PSUM"))
            ps_ot = ctx.enter_context(tc.tile_pool(name="ps_ot", bufs=1, space="PSUM"))

            # ---- one-time constants / weights ----
            wq = wpool.tile([P, NCC, C], WDT, tag="wq")
            wk = wpool.tile([P, NCC, C], WDT, tag="wk")
            wv = wpool.tile([P, NCC, C], WDT, tag="wv")
            wp = wpool.tile([P, NCC, C], WDT, tag="wp")
            w1 = wpool.tile([P, NCC, F1], WDT, tag="w1")
            w2 = wpool.tile([P, NFB, C], WDT, tag="w2")
            for t_, d_ in ((wq, wq_d), (wk, wk_d), (wv, wv_d), (wp, wp_d)):
                nc.sync.dma_start(out=t_, in_=d_[:, :].rearrange("(cc p) f -> p cc f", p=P))
            nc.sync.dma_start(out=w1, in_=w1_d[:, :].rearrange("(cc p) f -> p cc f", p=P))
            nc.sync.dma_start(out=w2, in_=w2_d[:, :].rearrange("(fc p) c -> p fc c", p=P))

            b1s = cpool.tile([P, NFB], F32, tag="b1s")
            nc.sync.dma_start(out=b1s, in_=b1_d[:, :])
            g1t = bl1t = g2t = bl2t = bpt = b2t = None
            if not skip_gb:
                g1t = cpool.tile([P, C], F32, tag="g1t")
                bl1t = cpool.tile([P, C], F32, tag="bl1t")
                g2t = cpool.tile([P, C], F32, tag="g2t")
                bl2t = cpool.tile([P, C], F32, tag="bl2t")
                for t_, d_ in ((g1t, g1_d), (bl1t, bl1_d), (g2t, g2_d),
                               (bl2t, bl2_d)):
                    nc.sync.dma_start(out=t_, in_=d_[:, :])
            if not skip_bias:
                bpt = cpool.tile([P, C], F32, tag="bpt")
                b2t = cpool.tile([P, C], F32, tag="b2t")
                for t_, d_ in ((bpt, bp_d), (b2t, b2_d)):
                    nc.sync.dma_start(out=t_, in_=d_[:, :])

            # maskT[s, t] = 0 if s <= t else -1e9 (upper-triangular keep,
            # for the transposed-score diag chunks)
            maskT = cpool.tile([P, P], F32, tag="maskT")
            nc.gpsimd.memset(maskT, 0.0)
            nc.gpsimd.affine_select(
                out=maskT, in_=maskT, compare_op=ALU.is_ge, fill=-1e9,
                base=0, pattern=[[1, P]], channel_multiplier=-1)
            ident = cpool.tile([P, P], WDT, tag="ident")
            make_identity(nc, ident[:, :])
            epst = cpool.tile([P, 1], F32, tag="epst")
            nc.vector.memset(epst, EPS)
            ones32 = cpool.tile([P, 32], BF16, tag="ones32")
            nc.vector.memset(ones32, 1.0)

            def layernorm(src_tiles, gt, bt, h_out):
                # src_tiles: [P, NTB, C] f32 (token-major); h_out same shape
                for tb in range(NTB):
                    stats = lnpool.tile([P, 6], F32, tag="stats")
                    mv = lnpool.tile([P, 2], F32, tag="mv")
                    nc.vector.bn_stats(out=stats, in_=src_tiles[:, tb, :])
                    nc.vector.bn_aggr(out=mv, in_=stats)
                    rstd = lnpool.tile([P, 1], F32, tag="rstd")
                    nc.scalar.activation(out=rstd, in_=mv[:, 1:2], func=AF.Sqrt,
                                         bias=epst, scale=1.0)
                    nc.vector.reciprocal(out=rstd, in_=rstd)
                    if skip_gb:
                        nc.vector.tensor_scalar(out=h_out[:, tb, :],
                                                in0=src_tiles[:, tb, :],
                                                scalar1=mv[:, 0:1], scalar2=rstd,
                                                op0=ALU.subtract, op1=ALU.mult)
                    else:
                        tmp = lnpool.tile([P, C], F32, tag="lntmp")
                        nc.vector.tensor_scalar(out=tmp, in0=src_tiles[:, tb, :],
                                                scalar1=mv[:, 0:1], scalar2=rstd,
                                                op0=ALU.subtract, op1=ALU.mult)
                        nc.vector.tensor_tensor(out=tmp, in0=tmp, in1=gt,
                                                op=ALU.mult)
                        nc.vector.tensor_tensor(out=h_out[:, tb, :], in0=tmp,
                                                in1=bt, op=ALU.add)

            def transpose_to(h_src, ht_out):
                # h_src [P, NTB, C] token-major -> ht_out [P, NCC, T]
                for cc in range(NCC):
                    tp = ps_pt.tile([P, T], WDT, tag="ptps")
                    for tb in range(NTB):
                        nc.tensor.transpose(out=tp[:, tb * P:(tb + 1) * P],
                                            in_=h_src[:, tb, cc * P:(cc + 1) * P],
                                            identity=ident)
                    xcopy(ht_out[:, cc, :], tp)

            copy_flip = [0]

            def xcopy(dst, src):
                # alternate PSUM->SBUF copies between ACT and DVE to balance
                if copy_flip[0] % 2 == 0:
                    nc.scalar.copy(out=dst, in_=src)
                else:
                    nc.vector.tensor_copy(out=dst, in_=src)
                copy_flip[0] += 1

            def contract_c(mm, w, fb, rhs):
                # mm += w[:, :, fb*P:(fb+1)*P].T @ rhs over all NCC chunks
                if use_fp8:
                    for cc in range(0, NCC, 2):
                        nc.tensor.matmul(
                            mm, lhsT=w[:, cc:cc + 2, fb * P:(fb + 1) * P],
                            rhs=rhs[:, cc:cc + 2, :],
                            start=(cc == 0), stop=(cc == NCC - 2),
                            perf_mode=DR)
                else:
                    for cc in range(NCC):
                        nc.tensor.matmul(
                            mm, lhsT=w[:, cc, fb * P:(fb + 1) * P],
                            rhs=rhs[:, cc, :],
                            start=(cc == 0), stop=(cc == NCC - 1))

            # ---- per batch element, software-pipelined ----
            # front/attn/tail are generators yielding at PE-chunk boundaries;
            # the driver round-robins them so the PE instruction stream
            # interleaves stall-prone attention matmuls with dense GEMMs
            # (keeps the HAM activity monitor warm and absorbs dependency
            # micro-waits -- throttle_active was 379us/43% without this).
            state = {}

            def transpose_gen(h_src, ht_out):
                for cc in range(NCC):
                    tp = ps_pt.tile([P, T], WDT, tag="ptps")
                    for tb in range(NTB):
                        nc.tensor.transpose(out=tp[:, tb * P:(tb + 1) * P],
                                            in_=h_src[:, tb, cc * P:(cc + 1) * P],
                                            identity=ident)
                    xcopy(ht_out[:, cc, :], tp)
                    yield

            def front(b):
                xt = xpool.tile([P, NTB, C], F32, tag="xt")
                nc.sync.dma_start(out=xt, in_=xs[b].rearrange("(tb p) c -> p tb c", p=P))
                h1 = hpool.tile([P, NTB, C], WDT, tag="h")
                layernorm(xt, g1t, bl1t, h1)
                yield
                h1t = htpool.tile([P, NCC, T], WDT, tag="ht")
                yield from transpose_gen(h1, h1t)
                qt = qpool.tile([P, NCC, T], BF16, tag="qt")
                kt = kpool.tile([P, NCC, T], BF16, tag="kt")
                state[b] = dict(xt=xt, qt=qt, kt=kt)
                for dst, w in ((qt, wq), (kt, wk)):
                    for fb in range(NCC):
                        mm = ps_mm.tile([P, T], F32, tag="mm")
                        contract_c(mm, w, fb, h1t)
                        xcopy(dst[:, fb, :], mm)
                        yield
                vt = vpool.tile([P, NTB, C], BF16, tag="vt")
                state[b]["vt"] = vt
                for tb in range(NTB):
                    mm = ps_mm.tile([P, C], F32, tag="mm")
                    if use_fp8:
                        for cc in range(0, NCC, 2):
                            nc.tensor.matmul(
                                mm, lhsT=h1t[:, cc:cc + 2, tb * P:(tb + 1) * P],
                                rhs=wv[:, cc:cc + 2, :],
                                start=(cc == 0), stop=(cc == NCC - 2),
                                perf_mode=DR)
                    else:
                        for cc in range(NCC):
                            nc.tensor.matmul(mm, lhsT=h1t[:, cc, tb * P:(tb + 1) * P],
                                             rhs=wv[:, cc, :],
                                             start=(cc == 0), stop=(cc == NCC - 1))
                    xcopy(vt[:, tb, :], mm)
                    yield

            def attn(b):
                st = state[b]
                qt, kt, vt, xt = st["qt"], st["kt"], st["vt"], st["xt"]
                otn = opool.tile([P, NCC, T], WDT, tag="otn")
                for g in range(NG):
                    # transposed scores + exp -> P^T chunks (s on partitions).
                    pth = {a: ptpool.tile([P, NTB, T], BF16, name=f"pth{a}",
                                          tag=f"pth{a}")
                           for a in range(4)}
                    # ones-matmul Z (replicated into 32-row bands) + O^T
                    # accumulation, both col-banded per head
                    ztp = ps_z.tile([P, T], F32, tag="ztp")
                    ot = ps_ot.tile([P, T], F32, tag="otps")

                    def zpv(sc):
                        for a in range(4):
                            nc.tensor.matmul(
                                ztp[32 * a:32 * (a + 1), sc * P:],
                                lhsT=ones32,
                                rhs=pth[a][:, sc, sc * P:],
                                start=(sc == 0), stop=(sc == NTB - 1),
                                tile_position=(0, 32 * a),
                                skip_group_check=True)
                            nc.tensor.matmul(
                                ot[32 * a:32 * (a + 1), sc * P:],
                                lhsT=vt[:, sc, 32 * (4 * g + a):32 * (4 * g + a + 1)],
                                rhs=pth[a][:, sc, sc * P:],
                                start=(sc == 0), stop=(sc == NTB - 1),
                                tile_position=(0, 32 * a),
                                skip_group_check=True)

                    for sc in range(NTB):
                        e2 = T - sc * P
                        for a in range(4):
                            sp = ps_mm.tile([P, T], F32, tag="mm")
                            nc.tensor.matmul(
                                sp[:, :e2],
                                lhsT=kt[32 * a:32 * (a + 1), g, sc * P:(sc + 1) * P],
                                rhs=qt[32 * a:32 * (a + 1), g, sc * P:],
                                start=True, stop=True,
                                tile_position=(32 * a, 0))
                            # off-diag exp does not depend on the mask-add
                            if e2 > P:
                                nc.scalar.activation(
                                    out=pth[a][:, sc, (sc + 1) * P:],
                                    in_=sp[:, P:e2], func=AF.Exp, scale=1.0)
                            nc.vector.tensor_tensor(out=sp[:, :P], in0=sp[:, :P],
                                                    in1=maskT, op=ALU.add)
                            nc.scalar.activation(out=pth[a][:, sc, sc * P:(sc + 1) * P],
                                                 in_=sp[:, :P],
                                                 func=AF.Exp, scale=1.0)
                            if a % 2 == 1:
                                yield
                        if sc > 0:
                            zpv(sc - 1)
                            yield
                    zpv(NTB - 1)
                    yield
                    # 1/Z = exp(-ln Z) on ACT (the DVE reciprocal on [128,512]
                    # costs ~3.4us and stalled the PE pipeline)
                    ztln = zpool.tile([P, T], F32, tag="ztln")
                    nc.scalar.activation(out=ztln, in_=ztp, func=AF.Ln)
                    ztr = zpool.tile([P, T], BF16, tag="ztr")
                    nc.scalar.activation(out=ztr, in_=ztln, func=AF.Exp,
                                         scale=-1.0)
                    nc.vector.tensor_tensor(out=otn[:, g, :], in0=ot, in1=ztr,
                                            op=ALU.mult)
                    yield
                # proj + residual 1
                r1 = rpool.tile([P, NTB, C], F32, tag="r1")
                state[b]["r1"] = r1
                for tb in range(NTB):
                    mm = ps_mm.tile([P, C], F32, tag="mm")
                    if use_fp8:
                        for fc in range(0, NCC, 2):
                            nc.tensor.matmul(
                                mm, lhsT=otn[:, fc:fc + 2, tb * P:(tb + 1) * P],
                                rhs=wp[:, fc:fc + 2, :],
                                start=(fc == 0), stop=(fc == NCC - 2),
                                perf_mode=DR)
                    else:
                        for fc in range(NCC):
                            nc.tensor.matmul(mm, lhsT=otn[:, fc, tb * P:(tb + 1) * P],
                                             rhs=wp[:, fc, :],
                                             start=(fc == 0), stop=(fc == NCC - 1))
                    if skip_bias:
                        nc.vector.tensor_tensor(out=r1[:, tb, :], in0=mm,
                                                in1=xt[:, tb, :], op=ALU.add)
                    else:
                        nc.vector.tensor_tensor(out=r1[:, tb, :], in0=mm,
                                                in1=bpt, op=ALU.add)
                        nc.vector.tensor_tensor(out=r1[:, tb, :],
                                                in0=r1[:, tb, :],
                                                in1=xt[:, tb, :], op=ALU.add)
                    yield

            def tail(b):
                r1 = state[b]["r1"]
                h2 = hpool.tile([P, NTB, C], WDT, tag="h")
                layernorm(r1, g2t, bl2t, h2)
                yield
                h2t = htpool.tile([P, NCC, T], WDT, tag="ht")
                yield from transpose_gen(h2, h2t)
                at = apool.tile([P, NFB, T], WDT, tag="at")
                for fb in range(NFB):
                    mm = ps_mm.tile([P, T], F32, tag="mm")
                    contract_c(mm, w1, fb, h2t)
                    if skip_bias and fb % 2 == 0:
                        nc.vector.tensor_scalar_max(at[:, fb, :], mm, 0.0)
                    else:
                        nc.scalar.activation(out=at[:, fb, :], in_=mm,
                                             func=AF.Relu,
                                             bias=b1s[:, fb:fb + 1], scale=1.0)
                    yield
                for tb in range(NTB):
                    mm = ps_mm.tile([P, C], F32, tag="mm")
                    if use_fp8:
                        for fc in range(0, NFB, 2):
                            nc.tensor.matmul(
                                mm, lhsT=at[:, fc:fc + 2, tb * P:(tb + 1) * P],
                                rhs=w2[:, fc:fc + 2, :],
                                start=(fc == 0), stop=(fc == NFB - 2),
                                perf_mode=DR)
                            if fc % 8 == 6:
                                yield
                    else:
                        for fc in range(NFB):
                            nc.tensor.matmul(mm, lhsT=at[:, fc, tb * P:(tb + 1) * P],
                                             rhs=w2[:, fc, :],
                                             start=(fc == 0), stop=(fc == NFB - 1))
                            if fc % 8 == 7 and fc != NFB - 1:
                                yield
                    ob = obpool.tile([P, C], F32, tag="ob")
                    nc.vector.tensor_tensor(out=ob, in0=mm, in1=r1[:, tb, :],
                                            op=ALU.add)
                    if not skip_bias:
                        nc.vector.tensor_tensor(out=ob, in0=ob, in1=b2t,
                                                op=ALU.add)
                    nc.sync.dma_start(
                        out=out_d[b].rearrange("(tb p) c -> p tb c", p=P)[:, tb, :],
                        in_=ob)
                    yield
                del state[b]

            def run_gen(g):
                for _ in g:
                    pass

            for b in range(bpc):
                run_gen(front(b))
                if b > 0:
                    run_gen(tail(b - 1))
                run_gen(attn(b))
            run_gen(tail(bpc - 1))
    return nc


_wsplit_ctr = [0]


def _split_multi_waits(nc):
    """walrus here rejects >1 sync wait per instruction; move extras onto
    standalone InstEventSemaphore carriers on the same engine."""
    for f in nc.m.functions:
        for b in f.blocks:
            insts = b.instructions
            if not any(i.sync_info and i.sync_info.on_wait and
                       len(i.sync_info.on_wait) > 1 for i in insts):
                continue
            new = []
            for inst in insts:
                si = inst.sync_info
                if si is not None and si.on_wait and len(si.on_wait) > 1:
                    waits = list(si.on_wait)
                    for w in waits[:-1]:
                        _wsplit_ctr[0] += 1
                        car = mybir.InstEventSemaphore(
                            name=f"W-split-{_wsplit_ctr[0]}", ins=[], outs=[])
                        car.engine = inst.engine
                        car.sync_info = mybir.SyncInfo(on_wait=[w], on_update=[])
                        new.append(car)
                    inst.sync_info = mybir.SyncInfo(
                        on_wait=[waits[-1]], on_update=list(si.on_update))
                new.append(inst)
            b.instructions = new
    return nc


_CACHED = {}


def _prep_inputs(inputs, use_fp8=USE_FP8):
    wdt = ml_dtypes.float8_e4m3 if use_fp8 else ml_dtypes.bfloat16
    s = D ** -0.5
    wq_p = (np.asarray(inputs["Wq"]).transpose(1, 0, 2).reshape(C, C) * s).astype(wdt)
    wk_p = np.asarray(inputs["Wk"]).transpose(1, 0, 2).reshape(C, C).astype(wdt)
    wv_p = np.asarray(inputs["Wv"]).transpose(1, 0, 2).reshape(C, C).astype(wdt)
    wp_p = np.asarray(inputs["Wp"]).astype(wdt)
    w1_p = np.asarray(inputs["W1"]).astype(wdt)
    w2_p = np.asarray(inputs["W2"]).astype(wdt)
    b1s = np.ascontiguousarray(
        np.asarray(inputs["b1"], dtype=np.float32).reshape(NFB, P).T)
    bc = lambda v: np.ascontiguousarray(
        np.broadcast_to(np.asarray(v, np.float32)[None, :], (P, C)))
    com = dict(wq=wq_p, wk=wk_p, wv=wv_p, wp=wp_p, w1=w1_p, w2=w2_p, b1s=b1s,
               g1=bc(inputs["g_ln1"]), bl1=bc(inputs["b_ln1"]),
               g2=bc(inputs["g_ln2"]), bl2=bc(inputs["b_ln2"]),
               bp=bc(inputs["bp"]), b2=bc(inputs["b2"]))
    x = np.asarray(inputs["x"], np.float32)
    in_maps = []
    for c in range(NCORES):
        m = dict(com)
        m["xs"] = np.ascontiguousarray(x[c * BPC:(c + 1) * BPC])
        in_maps.append(m)
    return in_maps


def _run(inputs, trace=False):
    skip_gb = (np.all(np.asarray(inputs["g_ln1"]) == 1) and
               np.all(np.asarray(inputs["g_ln2"]) == 1) and
               np.all(np.asarray(inputs["b_ln1"]) == 0) and
               np.all(np.asarray(inputs["b_ln2"]) == 0))
    skip_bias = (np.all(np.asarray(inputs["bp"]) == 0) and
                 np.all(np.asarray(inputs["b2"]) == 0))
    key = ("nc", skip_gb, skip_bias, USE_FP8)
    if key not in _CACHED:
        _CACHED[key] = _split_multi_waits(build_nc(skip_gb, skip_bias))
    nc = _CACHED[key]
    in_maps = _prep_inputs(inputs)
    res = run_bass_kernel_spmd(nc, in_maps, core_ids=list(range(NCORES)),
                               trace=trace)
    out = np.concatenate([r["out"] for r in res.results], axis=0)
    return out, res


def kernel(**inputs):
    out, _ = _run(inputs, trace=False)
    return out
